# revision 50
# baseline (speedup 1.0000x reference)
"""Trainium2 Bass kernel for a GQA sliding-window attention layer.

Reference computation (B=2, T=2048, C=2048, 16 Q heads / 4 KV heads, d=128):
    q = x @ Wq; k = x @ Wk; v = x @ Wv (+ sigmoid-gated value embedding)
    q, k = rmsnorm(rope(q)), rmsnorm(rope(k))
    scores masked to the band 0 <= j - i < window (=1024), softmax over j
    out = (p @ v) @ Wo

Sharding: 8 cores = 2 batches x 4 KV groups.  Each core computes its 4 Q
heads / 1 KV head for one batch and a partial output (its 512-row slice of
the Wo contraction); the host sums the 4 partials per batch.

Layout strategy per core:
  - xT (C x T, bf16) resident in SBUF; all projections contract over C.
  - q̂T / k̂T kept [d=128 partitions, T free]; scores computed transposed
    (S^T tiles [kj, qi]) so that P^T feeds the PV matmul directly with v in
    natural [token, d] layout (no P transposes).
  - softmax has no max-subtraction: rms-normalized q,k bound |score| by
    sqrt(128), so exp is safe in fp32.
  - per-q softmax denominators and rms rows are broadcast across partitions
    via a tiny DRAM bounce (SBUF APs need nonzero partition stride).
"""

import numpy as np
import ml_dtypes
from collections import deque

BF16 = ml_dtypes.bfloat16

# Problem dims (hardcoded per contest rules)
B, T, C = 2, 2048, 2048
N_HEAD, N_KV, HD, GATE_CH = 16, 4, 128, 32
WINDOW = 1024
P = 128
GH = N_HEAD // N_KV  # q heads per kv head (= per core)
N_CORES = 8

_PROGRAM_CACHE = {}


def build_program(T_=T, C_=C, win=WINDOW):
    import concourse.mybir as mybir
    import concourse.tile as tile
    from concourse import bacc

    dt = mybir.dt
    f32 = dt.float32
    bf16 = dt.bfloat16
    AF = mybir.ActivationFunctionType
    ALU = mybir.AluOpType

    NT = T_ // P          # token tiles
    KT = C_ // P          # contraction tiles
    WT = win // P         # window tiles
    ISQ = 1.0 / float(np.sqrt(HD))

    nc = bacc.Bacc()

    xT = nc.declare_dram_parameter("xT", [C_, T_], bf16, isOutput=False)
    wq = nc.declare_dram_parameter("wq", [C_, GH * HD], bf16, isOutput=False)
    wk = nc.declare_dram_parameter("wk", [C_, HD], bf16, isOutput=False)
    wv = nc.declare_dram_parameter("wv", [C_, HD], bf16, isOutput=False)
    wg = nc.declare_dram_parameter("wg", [GATE_CH, 1], bf16, isOutput=False)
    ve2 = nc.declare_dram_parameter("ve2", [T_, HD], bf16, isOutput=False)
    wo = nc.declare_dram_parameter("wo", [GH * HD, C_], bf16, isOutput=False)
    ccd = nc.declare_dram_parameter("cc", [P, T_], bf16, isOutput=False)
    ssd = nc.declare_dram_parameter("ss", [P, T_], bf16, isOutput=False)
    tlo = nc.declare_dram_parameter("tlo", [P, P], bf16, isOutput=False)
    thi = nc.declare_dram_parameter("thi", [P, P], bf16, isOutput=False)
    idr = nc.declare_dram_parameter("identr", [P, GH * P], bf16, isOutput=False)
    idf = nc.declare_dram_parameter("identf", [P, P], f32, isOutput=False)
    out_d = nc.declare_dram_parameter("out", [T_, C_], f32, isOutput=True)
    f32r = dt.float32r

    with tile.TileContext(nc) as tc:
        with (
            tc.tile_pool(name="singles", bufs=1) as sg,
            tc.tile_pool(name="work", bufs=2) as wk_pool,
            tc.tile_pool(name="work3", bufs=4) as w3_pool,
            tc.tile_pool(name="attw", bufs=4) as aw,
            tc.tile_pool(name="yup", bufs=3) as yu_pool,
            tc.tile_pool(name="outp", bufs=3) as op_pool,
            tc.tile_pool(name="psum", bufs=8, space="PSUM") as pp,
            tc.tile_pool(name="drb", bufs=4, space="DRAM") as drp,
        ):
            # ---- persistent inputs -------------------------------------
            # weight/x DMAs are split per k-tile and interleaved so the
            # first projection matmuls (kt=0) can start almost immediately
            # small constants FIRST (the rope tails read cc/ss early — they
            # must not queue behind the 13MB of x/weight traffic)
            wg_sb = sg.tile([GATE_CH, 1], bf16, tag="wg")
            nc.sync.dma_start(out=wg_sb[:], in_=wg[:])
            cc_sb = sg.tile([P, T_], bf16, tag="cc")
            nc.sync.dma_start(out=cc_sb[:], in_=ccd[:])
            ss_sb = sg.tile([P, T_], bf16, tag="ss")
            nc.sync.dma_start(out=ss_sb[:], in_=ssd[:])
            ve2_sb = sg.tile([P, NT, HD], bf16, tag="ve2")
            nc.sync.dma_start(out=ve2_sb[:], in_=ve2.rearrange("(o p) d -> p o d", p=P))
            tlo_sb = sg.tile([P, P], bf16, tag="tlo")
            nc.sync.dma_start(out=tlo_sb[:], in_=tlo[:])
            thi_sb = sg.tile([P, P], bf16, tag="thi")
            nc.sync.dma_start(out=thi_sb[:], in_=thi[:])
            idr_sb = sg.tile([P, GH * P], bf16, tag="idr")
            nc.sync.dma_start(out=idr_sb[:], in_=idr[:])
            idf_sb = sg.tile([P, P], f32, tag="idf")
            nc.sync.dma_start(out=idf_sb[:], in_=idf[:])
            xt = []
            wq_sb = sg.tile([P, KT, GH * HD], bf16, tag="wq")
            wk_sb = sg.tile([P, KT, HD], bf16, tag="wk")
            wv_sb = sg.tile([P, KT, HD], bf16, tag="wv")
            wqr = wq.rearrange("(o p) n -> p o n", p=P)
            wkr = wk.rearrange("(o p) n -> p o n", p=P)
            wvr = wv.rearrange("(o p) n -> p o n", p=P)
            for kt in range(KT):
                t_ = sg.tile([P, T_], bf16, tag=f"xt{kt}")
                nc.sync.dma_start(out=wk_sb[:, kt, :], in_=wkr[:, kt, :])
                nc.sync.dma_start(out=t_[:], in_=xT[kt * P:(kt + 1) * P, :])
                xt.append(t_)
                nc.sync.dma_start(out=wq_sb[:, kt, :], in_=wqr[:, kt, :])
                nc.sync.dma_start(out=wv_sb[:, kt, :], in_=wvr[:, kt, :])
            wo_sb = sg.tile([P, GH, C_], bf16, tag="wo")
            nc.sync.dma_start(out=wo_sb[:], in_=wo.rearrange("(o p) n -> p o n", p=P))
            ones_sb = sg.tile([P, 1], bf16, tag="onesb")
            nc.vector.memset(ones_sb[:], 1.0)
            ones1f = sg.tile([1, P], f32, tag="ones1f")
            nc.vector.memset(ones1f[:], 1.0)
            eps_sb = sg.tile([P, 1], f32, tag="epsb")
            nc.vector.memset(eps_sb[:], 1e-6)
            # for the K head: rms and the 1/sqrt(d) score scale fold into one
            # sqrt via sqrt(ssq + HD*eps) = sqrt(HD) * sqrt(mean+eps)
            eps2_sb = sg.tile([P, 1], f32, tag="eps2b")
            nc.vector.memset(eps2_sb[:], 1e-6 * HD)

            # persistent intermediates
            qhat = sg.tile([P, GH, T_], bf16, tag="qhat")   # normalized roped q, [d, h, t]
            khat = sg.tile([P, T_], bf16, tag="khat")       # normalized roped k * isq
            vsb = sg.tile([P, NT, HD], bf16, tag="vsb")     # gated v, [tok, tt, d]

            TS = T_ // 512  # 512-wide token slices

            # ---- projections + rope + rmsnorm for k/q heads and vT -----
            # Emitted as kt-major WAVES of 3 output groups: the PE chases the
            # xT DMAs tile-by-tile during the ramp, and each wave's dependent
            # tail work (rope/rms/broadcast) is batched behind the next
            # wave's matmuls so the PE stream never waits on DVE/ACT chains.
            def wave_mms(wave):
                items = []
                for (head, ts_) in wave:
                    sl = slice(ts_ * 512, ts_ * 512 + 512)
                    ps = pp.tile([P, 512], f32, tag="pb",
                                 name=f"ps{head}_{ts_}")
                    items.append((head, sl, ps))
                for kt in range(KT):
                    for gi, (head, ts_) in enumerate(wave):
                        if head == 0:
                            w_ap = wk_sb[:, kt, :]
                        elif head == GH + 1:
                            w_ap = wv_sb[:, kt, :]
                        else:
                            w_ap = wq_sb[:, kt, (head - 1) * HD:head * HD]
                        nc.tensor.matmul(
                            items[gi][2][:], lhsT=w_ap,
                            rhs=xt[kt][:, items[gi][1]],
                            start=(kt == 0), stop=(kt == KT - 1),
                        )
                return items

            def v_tail(head, sl, ps):
                # vT psum [d, tok] -> sbuf f32, then PE-transpose each 128-tok
                # block to natural [tok, d] and add the sigmoid-gated ve.
                vt = wk_pool.tile([P, 512], f32, tag="vt")
                nc.vector.tensor_copy(vt[:], ps[:])
                for i in range(4):
                    tt = sl.start // P + i
                    tsl = slice(tt * P, (tt + 1) * P)
                    tp = pp.tile([P, P], f32, tag="pb")
                    nc.tensor.transpose(tp[:], vt[:, i * P:(i + 1) * P], idf_sb[:])
                    gps = pp.tile([P, 1], f32, tag="pb")
                    nc.tensor.matmul(gps[:], lhsT=xt[0][0:GATE_CH, tsl],
                                     rhs=wg_sb[:], start=True, stop=True)
                    gcol = wk_pool.tile([P, 1], f32, tag="gcol")
                    nc.scalar.activation(gcol[:], gps[:], AF.Sigmoid)
                    # v = ve2 * sigmoid(g) + v_proj (ve2 pre-scaled by 2)
                    nc.vector.scalar_tensor_tensor(
                        out=vsb[:, tt, :], in0=ve2_sb[:, tt, :], scalar=gcol[:],
                        in1=tp[:], op0=ALU.mult, op1=ALU.add,
                    )

            def wave_tails(items):
                t1 = []
                for (head, sl, ps) in items:
                    if head == GH + 1:
                        v_tail(head, sl, ps)
                        continue
                    # rope: qr = ps*cc + swap(ps)*ss  (ss carries the sign)
                    qr = w3_pool.tile([P, 512], f32, tag="qr")
                    nc.vector.tensor_mul(qr[:], ps[:], cc_sb[:, sl])
                    qs = wk_pool.tile([P, 512], f32, tag="qs")
                    nc.vector.tensor_mul(qs[0:64, :], ps[64:128, :],
                                         ss_sb[0:64, sl])
                    nc.vector.tensor_mul(qs[64:128, :], ps[0:64, :],
                                         ss_sb[64:128, sl])
                    nc.vector.tensor_add(qr[:], qr[:], qs[:])
                    q2 = wk_pool.tile([P, 512], bf16, tag="q2")
                    nc.gpsimd.tensor_mul(q2[:], qr[:], qr[:])
                    t1.append((head, sl, qr, q2))
                ssqs = []
                for (head, sl, qr, q2) in t1:
                    ssq = pp.tile([1, 512], f32, tag="pb")
                    nc.tensor.matmul(ssq[:], lhsT=ones_sb[:], rhs=q2[:],
                                     start=True, stop=True)
                    ssqs.append(ssq)
                rows = []
                for (head, sl, qr, q2), ssq in zip(t1, ssqs):
                    srow = w3_pool.tile([1, 512], f32, tag="srow")
                    if head == 0:
                        nc.scalar.activation(srow[:], ssq[:], AF.Sqrt,
                                             bias=eps2_sb[0:1, :], scale=1.0)
                    else:
                        nc.scalar.activation(srow[:], ssq[:], AF.Sqrt,
                                             bias=eps_sb[0:1, :], scale=1.0 / HD)
                    rows.append(srow)
                rrs = []
                for (head, sl, qr, q2), srow in zip(t1, rows):
                    rr = w3_pool.tile([1, 512], f32, tag="rr")
                    nc.vector.reciprocal_approx_fast(rr[:], srow[:])
                    rrs.append(rr)
                rrbs = []
                for (head, sl, qr, q2), rr in zip(t1, rrs):
                    # broadcast across partitions via a tiny DRAM bounce —
                    # zero engine cost, latency hidden by the wave pipeline
                    db = drp.tile([1, 512], f32, tag="bounce")
                    nc.sync.dma_start(out=db[:], in_=rr[:])
                    rrb = w3_pool.tile([P, 512], f32, tag="rrb")
                    nc.sync.dma_start(out=rrb[:], in_=db[:].to_broadcast((P, 512)))
                    rrbs.append(rrb)
                for (head, sl, qr, q2), rrb in zip(t1, rrbs):
                    dest = khat[:, sl] if head == 0 else qhat[:, head - 1, sl]
                    nc.vector.tensor_mul(dest, qr[:], rrb[:])

            groups = [(head, ts_) for head in range(GH + 2)
                      for ts_ in range(TS)]
            prev_items = None
            for w0 in range(0, len(groups), 3):
                items = wave_mms(groups[w0:w0 + 3])
                if prev_items:
                    wave_tails(prev_items)
                prev_items = items
            wave_tails(prev_items)

            CO = C_ // 512  # output column chunks
            # All 4 q-heads are fused into one 512-wide moving operand:
            # scores / exp / den / PV are each ONE N=512 instruction per
            # (qi, kt), so LDWEIGHTS fully hides under the matmul stream.
            denps = {}
            yps = {}
            yus = {}
            rds = {}

            def attn_scores_k(qi, kk):
                ktc = min(WT + 1, NT - qi)
                qs4 = qhat[:, :, qi * P:(qi + 1) * P]   # [d, (h, q)] = 512 wide
                kt = qi + kk
                sp = pp.tile([P, GH * P], f32, tag="pb")
                masked = (kk == 0) or (kk == WT and ktc == WT + 1)
                nc.tensor.matmul(
                    sp[:], lhsT=khat[:, kt * P:(kt + 1) * P], rhs=qs4,
                    start=True, stop=not masked,
                )
                if masked:
                    # band-mask bias (-3e4 outside band): psum += bias.T @ I_rep
                    nc.tensor.matmul(
                        sp[:], lhsT=tlo_sb[:] if kk == 0 else thi_sb[:],
                        rhs=idr_sb[:], start=False, stop=True,
                    )
                pt = aw.tile([P, GH * P], bf16, tag="pT")
                nc.scalar.activation(pt[:], sp[:], AF.Exp)
                return pt

            def attn_pv_k(qi, kk, pt):
                ktc = min(WT + 1, NT - qi)
                if kk == 0:
                    denps[qi] = pp.tile([1, GH * P], f32, tag="pb",
                                        name=f"denp{qi}")
                    yps[qi] = pp.tile([P, GH * P], f32, tag="pb",
                                      name=f"yp{qi}")
                kt = qi + kk
                nc.tensor.matmul(
                    denps[qi][:], lhsT=ones_sb[:], rhs=pt[:],
                    start=(kk == 0), stop=(kk == ktc - 1),
                )
                nc.tensor.matmul(
                    yps[qi][:], lhsT=vsb[:, kt, :], rhs=pt[:],
                    start=(kk == 0), stop=(kk == ktc - 1),
                )
                if kk == ktc - 1:
                    yut = yu_pool.tile([P, GH * P], f32, tag="yu")
                    nc.vector.tensor_copy(yut[:], yps[qi][:])
                    yus[qi] = yut
                    rd = wk_pool.tile([1, GH * P], f32, tag="rd")
                    nc.vector.reciprocal_approx_fast(rd[:], denps[qi][:])
                    rds[qi] = rd

            def attn_out(qi):
                qsl = slice(qi * P, (qi + 1) * P)
                rdb = pp.tile([P, GH * P], f32, tag="pb")
                nc.tensor.matmul(rdb[:], lhsT=ones1f[:], rhs=rds[qi][:],
                                 start=True, stop=True)
                yq = op_pool.tile([P, GH * P], bf16, tag="yq")
                nc.vector.tensor_mul(yq[:], yus[qi][:], rdb[:])
                for co in range(CO):
                    osl = slice(co * 512, co * 512 + 512)
                    ops = pp.tile([P, 512], f32, tag="pb")
                    for h in range(GH):
                        nc.tensor.matmul(
                            ops[:], lhsT=yq[:, h * P:(h + 1) * P],
                            rhs=wo_sb[:, h, osl],
                            start=(h == 0), stop=(h == GH - 1),
                        )
                    ob = op_pool.tile([P, 512], f32, tag="ob")
                    nc.vector.tensor_copy(out=ob[:], in_=ops[:])
                    nc.sync.dma_start(out=out_d[qsl, osl], in_=ob[:])

            pv_queue = deque()
            done_out = set()
            out_ready = deque()
            for qi in range(NT):
                ktc = min(WT + 1, NT - qi)
                for kk in range(ktc):
                    pt = attn_scores_k(qi, kk)
                    if len(pv_queue) >= 2:
                        attn_pv_k(*pv_queue.popleft())
                    pv_queue.append((qi, kk, pt))
                    # emit out-proj one iteration after its recip is queued,
                    # so the PE never waits on the denominator chain
                    if out_ready and out_ready[0][1] <= 0:
                        done_out.add(out_ready[0][0])
                        attn_out(out_ready.popleft()[0])
                    out_ready = deque([(q, age - 1) for q, age in out_ready])
                    if qi > 0 and (qi - 1) in rds and (qi - 1) not in done_out \
                            and all(q != qi - 1 for q, _ in out_ready):
                        out_ready.append((qi - 1, 1))
            while pv_queue:
                attn_pv_k(*pv_queue.popleft())
            for qi in range(NT):
                if qi not in done_out:
                    attn_out(qi)

    return nc


def _get_program(T_=T, C_=C, win=WINDOW):
    key = (T_, C_, win)
    if key not in _PROGRAM_CACHE:
        nc = build_program(T_, C_, win)
        nc.finalize()
        _PROGRAM_CACHE[key] = nc
    return _PROGRAM_CACHE[key]


def make_in_maps(x, ve, cos, sin, Wq, Wk, Wv, Wg, Wo):
    """Build the 8 per-core input dicts (host-side sharding/layout prep)."""
    cosT = np.ascontiguousarray(cos[:, 0, :].T).astype(np.float32)  # [64, T]
    sinT = np.ascontiguousarray(sin[:, 0, :].T).astype(np.float32)
    cc = np.concatenate([cosT, cosT], axis=0)            # [128, T]
    ss = np.concatenate([sinT, -sinT], axis=0)           # [128, T]
    # additive mask biases for the S^T diagonal/far tiles, pre-transposed
    # (they enter the scores as lhsT with an identity rhs: psum += bias.T)
    neg = np.float32(-30000.0)
    bias_lo = np.where(np.arange(P)[:, None] >= np.arange(P)[None, :], 0.0, neg)
    bias_hi = np.where(np.arange(P)[:, None] < np.arange(P)[None, :], 0.0, neg)
    tlo = np.ascontiguousarray(bias_lo.T).astype(BF16)
    thi = np.ascontiguousarray(bias_hi.T).astype(BF16)
    identr = np.tile(np.eye(P, dtype=np.float32), (1, GH)).astype(BF16)
    identf = np.eye(P, dtype=np.float32)

    in_maps = []
    for core in range(N_CORES):
        b, g = divmod(core, N_KV)
        in_maps.append({
            "xT": np.ascontiguousarray(x[b].T).astype(BF16),
            "wq": Wq[:, g * GH * HD:(g + 1) * GH * HD].astype(BF16),
            "wk": Wk[:, g * HD:(g + 1) * HD].astype(BF16),
            "wv": Wv[:, g * HD:(g + 1) * HD].astype(BF16),
            "wg": np.ascontiguousarray(Wg[:, g:g + 1]).astype(BF16),
            "ve2": (2.0 * ve[b][:, g * HD:(g + 1) * HD]).astype(BF16),
            "wo": Wo[g * GH * HD:(g + 1) * GH * HD, :].astype(BF16),
            "cc": cc.astype(BF16), "ss": ss.astype(BF16),
            "tlo": tlo, "thi": thi, "identr": identr, "identf": identf,
        })
    return in_maps


def kernel(x, ve, cos, sin, Wq, Wk, Wv, Wg, Wo, window):
    assert int(window) == WINDOW and x.shape == (B, T, C)
    from concourse.bass_utils import run_bass_kernel_spmd

    nc = _get_program()
    in_maps = make_in_maps(x, ve, cos, sin, Wq, Wk, Wv, Wg, Wo)
    res = run_bass_kernel_spmd(nc, in_maps, core_ids=list(range(N_CORES)))
    out = np.zeros((B, T, C), dtype=np.float32)
    for core in range(N_CORES):
        b = core // N_KV
        out[b] += res.results[core]["out"]
    return out


# revision 52
# speedup vs baseline: 1.0235x; 1.0235x over previous
"""Trainium2 Bass kernel for a GQA sliding-window attention layer.

Reference computation (B=2, T=2048, C=2048, 16 Q heads / 4 KV heads, d=128):
    q = x @ Wq; k = x @ Wk; v = x @ Wv (+ sigmoid-gated value embedding)
    q, k = rmsnorm(rope(q)), rmsnorm(rope(k))
    scores masked to the band 0 <= j - i < window (=1024), softmax over j
    out = (p @ v) @ Wo

Sharding: 8 cores = 2 batches x 4 KV groups.  Each core computes its 4 Q
heads / 1 KV head for one batch and a partial output (its 512-row slice of
the Wo contraction); the host sums the 4 partials per batch.

Layout strategy per core:
  - xT (C x T, bf16) resident in SBUF; all projections contract over C.
  - q̂T / k̂T kept [d=128 partitions, T free]; scores computed transposed
    (S^T tiles [kj, qi]) so that P^T feeds the PV matmul directly with v in
    natural [token, d] layout (no P transposes).
  - softmax has no max-subtraction: rms-normalized q,k bound |score| by
    sqrt(128), so exp is safe in fp32.
  - per-q softmax denominators and rms rows are broadcast across partitions
    via a tiny DRAM bounce (SBUF APs need nonzero partition stride).
"""

import numpy as np
import ml_dtypes
from collections import deque

BF16 = ml_dtypes.bfloat16

# Problem dims (hardcoded per contest rules)
B, T, C = 2, 2048, 2048
N_HEAD, N_KV, HD, GATE_CH = 16, 4, 128, 32
WINDOW = 1024
P = 128
GH = N_HEAD // N_KV  # q heads per kv head (= per core)
N_CORES = 8

_PROGRAM_CACHE = {}


def build_program(T_=T, C_=C, win=WINDOW):
    import concourse.mybir as mybir
    import concourse.tile as tile
    from concourse import bacc

    dt = mybir.dt
    f32 = dt.float32
    bf16 = dt.bfloat16
    AF = mybir.ActivationFunctionType
    ALU = mybir.AluOpType

    NT = T_ // P          # token tiles
    KT = C_ // P          # contraction tiles
    WT = win // P         # window tiles
    ISQ = 1.0 / float(np.sqrt(HD))

    nc = bacc.Bacc()

    xT = nc.declare_dram_parameter("xT", [C_, T_], bf16, isOutput=False)
    wq = nc.declare_dram_parameter("wq", [C_, GH * HD], bf16, isOutput=False)
    wk = nc.declare_dram_parameter("wk", [C_, HD], bf16, isOutput=False)
    wv = nc.declare_dram_parameter("wv", [C_, HD], bf16, isOutput=False)
    wg = nc.declare_dram_parameter("wg", [GATE_CH, 1], bf16, isOutput=False)
    ve2 = nc.declare_dram_parameter("ve2", [T_, HD], bf16, isOutput=False)
    wo = nc.declare_dram_parameter("wo", [GH * HD, C_], bf16, isOutput=False)
    ccd = nc.declare_dram_parameter("cc", [P, T_], bf16, isOutput=False)
    ssd = nc.declare_dram_parameter("ss", [P, T_], bf16, isOutput=False)
    tlo = nc.declare_dram_parameter("tlo", [P, P], bf16, isOutput=False)
    thi = nc.declare_dram_parameter("thi", [P, P], bf16, isOutput=False)
    idr = nc.declare_dram_parameter("identr", [P, GH * P], bf16, isOutput=False)
    idf = nc.declare_dram_parameter("identf", [P, P], f32, isOutput=False)
    out_d = nc.declare_dram_parameter("out", [T_, C_], f32, isOutput=True)
    f32r = dt.float32r

    with tile.TileContext(nc) as tc:
        with (
            tc.tile_pool(name="singles", bufs=1) as sg,
            tc.tile_pool(name="work", bufs=2) as wk_pool,
            tc.tile_pool(name="work3", bufs=4) as w3_pool,
            tc.tile_pool(name="attw", bufs=4) as aw,
            tc.tile_pool(name="yup", bufs=3) as yu_pool,
            tc.tile_pool(name="outp", bufs=3) as op_pool,
            tc.tile_pool(name="psum", bufs=8, space="PSUM") as pp,
        ):
            # ---- persistent inputs -------------------------------------
            # weight/x DMAs are split per k-tile and interleaved so the
            # first projection matmuls (kt=0) can start almost immediately
            # small constants FIRST (the rope tails read cc/ss early — they
            # must not queue behind the 13MB of x/weight traffic)
            wg_sb = sg.tile([GATE_CH, 1], bf16, tag="wg")
            nc.sync.dma_start(out=wg_sb[:], in_=wg[:])
            cc_sb = sg.tile([P, T_], bf16, tag="cc")
            nc.sync.dma_start(out=cc_sb[:], in_=ccd[:])
            ss_sb = sg.tile([P, T_], bf16, tag="ss")
            nc.sync.dma_start(out=ss_sb[:], in_=ssd[:])
            ve2_sb = sg.tile([P, NT, HD], bf16, tag="ve2")
            nc.sync.dma_start(out=ve2_sb[:], in_=ve2.rearrange("(o p) d -> p o d", p=P))
            tlo_sb = sg.tile([P, P], bf16, tag="tlo")
            nc.sync.dma_start(out=tlo_sb[:], in_=tlo[:])
            thi_sb = sg.tile([P, P], bf16, tag="thi")
            nc.sync.dma_start(out=thi_sb[:], in_=thi[:])
            idr_sb = sg.tile([P, GH * P], bf16, tag="idr")
            nc.sync.dma_start(out=idr_sb[:], in_=idr[:])
            idf_sb = sg.tile([P, P], f32, tag="idf")
            nc.sync.dma_start(out=idf_sb[:], in_=idf[:])
            xt = []
            wq_sb = sg.tile([P, KT, GH * HD], bf16, tag="wq")
            wk_sb = sg.tile([P, KT, HD], bf16, tag="wk")
            wv_sb = sg.tile([P, KT, HD], bf16, tag="wv")
            wqr = wq.rearrange("(o p) n -> p o n", p=P)
            wkr = wk.rearrange("(o p) n -> p o n", p=P)
            wvr = wv.rearrange("(o p) n -> p o n", p=P)
            for kt in range(KT):
                t_ = sg.tile([P, T_], bf16, tag=f"xt{kt}")
                nc.sync.dma_start(out=wk_sb[:, kt, :], in_=wkr[:, kt, :])
                # per-512-col chunks: early waves depend only on slices read
                for c0 in range(0, T_, 512):
                    csl = slice(c0, c0 + 512)
                    nc.sync.dma_start(out=t_[:, csl],
                                      in_=xT[kt * P:(kt + 1) * P, csl])
                xt.append(t_)
                nc.sync.dma_start(out=wq_sb[:, kt, :], in_=wqr[:, kt, :])
                nc.sync.dma_start(out=wv_sb[:, kt, :], in_=wvr[:, kt, :])
            wo_sb = sg.tile([P, GH, C_], bf16, tag="wo")
            nc.sync.dma_start(out=wo_sb[:], in_=wo.rearrange("(o p) n -> p o n", p=P))
            ones_sb = sg.tile([P, 1], bf16, tag="onesb")
            nc.vector.memset(ones_sb[:], 1.0)
            ones1f = sg.tile([1, P], f32, tag="ones1f")
            nc.vector.memset(ones1f[:], 1.0)
            eps_sb = sg.tile([P, 1], f32, tag="epsb")
            nc.vector.memset(eps_sb[:], 1e-6)

            # persistent intermediates
            qhat = sg.tile([P, GH, T_], bf16, tag="qhat")   # normalized roped q, [d, h, t]
            khat = sg.tile([P, T_], bf16, tag="khat")       # normalized roped k * isq
            vsb = sg.tile([P, NT, HD], bf16, tag="vsb")     # gated v, [tok, tt, d]

            TS = T_ // 512  # 512-wide token slices

            # ---- projections + rope + rmsnorm for k/q heads and vT -----
            # Emitted as kt-major WAVES of 3 output groups: the PE chases the
            # xT DMAs tile-by-tile during the ramp, and each wave's dependent
            # tail work (rope/rms/broadcast) is batched behind the next
            # wave's matmuls so the PE stream never waits on DVE/ACT chains.
            def wave_mms(wave):
                items = []
                for (head, ts_) in wave:
                    sl = slice(ts_ * 512, ts_ * 512 + 512)
                    ps = pp.tile([P, 512], f32, tag="pb",
                                 name=f"ps{head}_{ts_}")
                    items.append((head, sl, ps))
                for kt in range(KT):
                    for gi, (head, ts_) in enumerate(wave):
                        if head == 0:
                            w_ap = wk_sb[:, kt, :]
                        elif head == GH + 1:
                            w_ap = wv_sb[:, kt, :]
                        else:
                            w_ap = wq_sb[:, kt, (head - 1) * HD:head * HD]
                        nc.tensor.matmul(
                            items[gi][2][:], lhsT=w_ap,
                            rhs=xt[kt][:, items[gi][1]],
                            start=(kt == 0), stop=(kt == KT - 1),
                        )
                return items

            def v_tail(head, sl, ps):
                # vT psum [d, tok] -> sbuf f32, then PE-transpose each 128-tok
                # block to natural [tok, d] and add the sigmoid-gated ve.
                vt = wk_pool.tile([P, 512], f32, tag="vt")
                nc.vector.tensor_copy(vt[:], ps[:])
                for i in range(4):
                    tt = sl.start // P + i
                    tsl = slice(tt * P, (tt + 1) * P)
                    tp = pp.tile([P, P], f32, tag="pb")
                    nc.tensor.transpose(tp[:], vt[:, i * P:(i + 1) * P], idf_sb[:])
                    gps = pp.tile([P, 1], f32, tag="pb")
                    nc.tensor.matmul(gps[:], lhsT=xt[0][0:GATE_CH, tsl],
                                     rhs=wg_sb[:], start=True, stop=True)
                    gcol = wk_pool.tile([P, 1], f32, tag="gcol")
                    nc.scalar.activation(gcol[:], gps[:], AF.Sigmoid)
                    # v = ve2 * sigmoid(g) + v_proj (ve2 pre-scaled by 2)
                    nc.vector.scalar_tensor_tensor(
                        out=vsb[:, tt, :], in0=ve2_sb[:, tt, :], scalar=gcol[:],
                        in1=tp[:], op0=ALU.mult, op1=ALU.add,
                    )

            def wave_tails(items):
                t1 = []
                for (head, sl, ps) in items:
                    if head == GH + 1:
                        v_tail(head, sl, ps)
                        continue
                    # rope: qr = ps*cc + swap(ps)*ss  (ss carries the sign)
                    qr = w3_pool.tile([P, 512], f32, tag="qr")
                    nc.vector.tensor_mul(qr[:], ps[:], cc_sb[:, sl])
                    qs = wk_pool.tile([P, 512], f32, tag="qs")
                    nc.vector.tensor_mul(qs[0:64, :], ps[64:128, :],
                                         ss_sb[0:64, sl])
                    nc.vector.tensor_mul(qs[64:128, :], ps[0:64, :],
                                         ss_sb[64:128, sl])
                    nc.vector.tensor_add(qr[:], qr[:], qs[:])
                    q2 = wk_pool.tile([P, 512], bf16, tag="q2")
                    nc.gpsimd.tensor_mul(q2[:], qr[:], qr[:])
                    t1.append((head, sl, qr, q2))
                ssqs = []
                for (head, sl, qr, q2) in t1:
                    ssq = pp.tile([1, 512], f32, tag="pb")
                    nc.tensor.matmul(ssq[:], lhsT=ones_sb[:], rhs=q2[:],
                                     start=True, stop=True)
                    ssqs.append(ssq)
                rows = []
                for (head, sl, qr, q2), ssq in zip(t1, ssqs):
                    srow = w3_pool.tile([1, 512], f32, tag="srow")
                    nc.scalar.activation(srow[:], ssq[:], AF.Sqrt,
                                         bias=eps_sb[0:1, :], scale=1.0 / HD)
                    rows.append(srow)
                rrs = []
                for (head, sl, qr, q2), srow in zip(t1, rows):
                    rr = w3_pool.tile([1, 512], f32, tag="rr")
                    nc.vector.reciprocal_approx_fast(rr[:], srow[:])
                    if head == 0:
                        # fold the 1/sqrt(d) score scale into k-hat
                        nc.vector.tensor_scalar_mul(rr[:], rr[:], ISQ)
                    rrs.append(rr)
                rrbs = []
                for (head, sl, qr, q2), rr in zip(t1, rrs):
                    rrb = pp.tile([P, 512], f32, tag="pb")
                    nc.tensor.matmul(rrb[:], lhsT=ones1f[:], rhs=rr[:],
                                     start=True, stop=True)
                    rrbs.append(rrb)
                for (head, sl, qr, q2), rrb in zip(t1, rrbs):
                    dest = khat[:, sl] if head == 0 else qhat[:, head - 1, sl]
                    nc.vector.tensor_mul(dest, qr[:], rrb[:])

            groups = [(head, ts_) for head in range(GH + 2)
                      for ts_ in range(TS)]
            prev_items = None
            for w0 in range(0, len(groups), 3):
                items = wave_mms(groups[w0:w0 + 3])
                if prev_items:
                    wave_tails(prev_items)
                prev_items = items
            wave_tails(prev_items)

            CO = C_ // 512  # output column chunks
            # All 4 q-heads are fused into one 512-wide moving operand:
            # scores / exp / den / PV are each ONE N=512 instruction per
            # (qi, kt), so LDWEIGHTS fully hides under the matmul stream.
            denps = {}
            yps = {}
            yus = {}
            rds = {}

            def attn_scores_k(qi, kk):
                ktc = min(WT + 1, NT - qi)
                qs4 = qhat[:, :, qi * P:(qi + 1) * P]   # [d, (h, q)] = 512 wide
                kt = qi + kk
                sp = pp.tile([P, GH * P], f32, tag="pb")
                masked = (kk == 0) or (kk == WT and ktc == WT + 1)
                nc.tensor.matmul(
                    sp[:], lhsT=khat[:, kt * P:(kt + 1) * P], rhs=qs4,
                    start=True, stop=not masked,
                )
                if masked:
                    # band-mask bias (-3e4 outside band): psum += bias.T @ I_rep
                    nc.tensor.matmul(
                        sp[:], lhsT=tlo_sb[:] if kk == 0 else thi_sb[:],
                        rhs=idr_sb[:], start=False, stop=True,
                    )
                pt = aw.tile([P, GH * P], bf16, tag="pT")
                nc.scalar.activation(pt[:], sp[:], AF.Exp)
                return pt

            def attn_pv_k(qi, kk, pt):
                ktc = min(WT + 1, NT - qi)
                if kk == 0:
                    denps[qi] = pp.tile([1, GH * P], f32, tag="pb",
                                        name=f"denp{qi}")
                    yps[qi] = pp.tile([P, GH * P], f32, tag="pb",
                                      name=f"yp{qi}")
                kt = qi + kk
                nc.tensor.matmul(
                    denps[qi][:], lhsT=ones_sb[:], rhs=pt[:],
                    start=(kk == 0), stop=(kk == ktc - 1),
                )
                nc.tensor.matmul(
                    yps[qi][:], lhsT=vsb[:, kt, :], rhs=pt[:],
                    start=(kk == 0), stop=(kk == ktc - 1),
                )
                if kk == ktc - 1:
                    yut = yu_pool.tile([P, GH * P], f32, tag="yu")
                    nc.vector.tensor_copy(yut[:], yps[qi][:])
                    yus[qi] = yut
                    rd = wk_pool.tile([1, GH * P], f32, tag="rd")
                    nc.vector.reciprocal_approx_fast(rd[:], denps[qi][:])
                    rds[qi] = rd

            def attn_out(qi):
                qsl = slice(qi * P, (qi + 1) * P)
                rdb = pp.tile([P, GH * P], f32, tag="pb")
                nc.tensor.matmul(rdb[:], lhsT=ones1f[:], rhs=rds[qi][:],
                                 start=True, stop=True)
                yq = op_pool.tile([P, GH * P], bf16, tag="yq")
                nc.vector.tensor_mul(yq[:], yus[qi][:], rdb[:])
                for co in range(CO):
                    osl = slice(co * 512, co * 512 + 512)
                    ops = pp.tile([P, 512], f32, tag="pb")
                    for h in range(GH):
                        nc.tensor.matmul(
                            ops[:], lhsT=yq[:, h * P:(h + 1) * P],
                            rhs=wo_sb[:, h, osl],
                            start=(h == 0), stop=(h == GH - 1),
                        )
                    ob = op_pool.tile([P, 512], f32, tag="ob")
                    nc.vector.tensor_copy(out=ob[:], in_=ops[:])
                    nc.sync.dma_start(out=out_d[qsl, osl], in_=ob[:])

            pv_queue = deque()
            done_out = set()
            out_ready = deque()
            for qi in range(NT):
                ktc = min(WT + 1, NT - qi)
                for kk in range(ktc):
                    pt = attn_scores_k(qi, kk)
                    if len(pv_queue) >= 2:
                        attn_pv_k(*pv_queue.popleft())
                    pv_queue.append((qi, kk, pt))
                    # emit out-proj one iteration after its recip is queued,
                    # so the PE never waits on the denominator chain
                    if out_ready and out_ready[0][1] <= 0:
                        done_out.add(out_ready[0][0])
                        attn_out(out_ready.popleft()[0])
                    out_ready = deque([(q, age - 1) for q, age in out_ready])
                    if qi > 0 and (qi - 1) in rds and (qi - 1) not in done_out \
                            and all(q != qi - 1 for q, _ in out_ready):
                        out_ready.append((qi - 1, 1))
            while pv_queue:
                attn_pv_k(*pv_queue.popleft())
            for qi in range(NT):
                if qi not in done_out:
                    attn_out(qi)

    return nc


def _get_program(T_=T, C_=C, win=WINDOW):
    key = (T_, C_, win)
    if key not in _PROGRAM_CACHE:
        nc = build_program(T_, C_, win)
        nc.finalize()
        _PROGRAM_CACHE[key] = nc
    return _PROGRAM_CACHE[key]


def make_in_maps(x, ve, cos, sin, Wq, Wk, Wv, Wg, Wo):
    """Build the 8 per-core input dicts (host-side sharding/layout prep)."""
    cosT = np.ascontiguousarray(cos[:, 0, :].T).astype(np.float32)  # [64, T]
    sinT = np.ascontiguousarray(sin[:, 0, :].T).astype(np.float32)
    cc = np.concatenate([cosT, cosT], axis=0)            # [128, T]
    ss = np.concatenate([sinT, -sinT], axis=0)           # [128, T]
    # additive mask biases for the S^T diagonal/far tiles, pre-transposed
    # (they enter the scores as lhsT with an identity rhs: psum += bias.T)
    neg = np.float32(-30000.0)
    bias_lo = np.where(np.arange(P)[:, None] >= np.arange(P)[None, :], 0.0, neg)
    bias_hi = np.where(np.arange(P)[:, None] < np.arange(P)[None, :], 0.0, neg)
    tlo = np.ascontiguousarray(bias_lo.T).astype(BF16)
    thi = np.ascontiguousarray(bias_hi.T).astype(BF16)
    identr = np.tile(np.eye(P, dtype=np.float32), (1, GH)).astype(BF16)
    identf = np.eye(P, dtype=np.float32)

    in_maps = []
    for core in range(N_CORES):
        b, g = divmod(core, N_KV)
        in_maps.append({
            "xT": np.ascontiguousarray(x[b].T).astype(BF16),
            "wq": Wq[:, g * GH * HD:(g + 1) * GH * HD].astype(BF16),
            "wk": Wk[:, g * HD:(g + 1) * HD].astype(BF16),
            "wv": Wv[:, g * HD:(g + 1) * HD].astype(BF16),
            "wg": np.ascontiguousarray(Wg[:, g:g + 1]).astype(BF16),
            "ve2": (2.0 * ve[b][:, g * HD:(g + 1) * HD]).astype(BF16),
            "wo": Wo[g * GH * HD:(g + 1) * GH * HD, :].astype(BF16),
            "cc": cc.astype(BF16), "ss": ss.astype(BF16),
            "tlo": tlo, "thi": thi, "identr": identr, "identf": identf,
        })
    return in_maps


def kernel(x, ve, cos, sin, Wq, Wk, Wv, Wg, Wo, window):
    assert int(window) == WINDOW and x.shape == (B, T, C)
    from concourse.bass_utils import run_bass_kernel_spmd

    nc = _get_program()
    in_maps = make_in_maps(x, ve, cos, sin, Wq, Wk, Wv, Wg, Wo)
    res = run_bass_kernel_spmd(nc, in_maps, core_ids=list(range(N_CORES)))
    out = np.zeros((B, T, C), dtype=np.float32)
    for core in range(N_CORES):
        b = core // N_KV
        out[b] += res.results[core]["out"]
    return out


# revision 53
# speedup vs baseline: 1.1069x; 1.0815x over previous
"""Trainium2 Bass kernel for a GQA sliding-window attention layer.

Reference computation (B=2, T=2048, C=2048, 16 Q heads / 4 KV heads, d=128):
    q = x @ Wq; k = x @ Wk; v = x @ Wv (+ sigmoid-gated value embedding)
    q, k = rmsnorm(rope(q)), rmsnorm(rope(k))
    scores masked to the band 0 <= j - i < window (=1024), softmax over j
    out = (p @ v) @ Wo

Sharding: 8 cores = 2 batches x 4 KV groups.  Each core computes its 4 Q
heads / 1 KV head for one batch and a partial output (its 512-row slice of
the Wo contraction); the host sums the 4 partials per batch.

Layout strategy per core:
  - xT (C x T, bf16) resident in SBUF; all projections contract over C.
  - q̂T / k̂T kept [d=128 partitions, T free]; scores computed transposed
    (S^T tiles [kj, qi]) so that P^T feeds the PV matmul directly with v in
    natural [token, d] layout (no P transposes).
  - softmax has no max-subtraction: rms-normalized q,k bound |score| by
    sqrt(128), so exp is safe in fp32.
  - per-q softmax denominators and rms rows are broadcast across partitions
    via a tiny DRAM bounce (SBUF APs need nonzero partition stride).
"""

import numpy as np
import ml_dtypes
from collections import deque

BF16 = ml_dtypes.bfloat16

# Problem dims (hardcoded per contest rules)
B, T, C = 2, 2048, 2048
N_HEAD, N_KV, HD, GATE_CH = 16, 4, 128, 32
WINDOW = 1024
P = 128
GH = N_HEAD // N_KV  # q heads per kv head (= per core)
N_CORES = 8

_PROGRAM_CACHE = {}


def build_program(T_=T, C_=C, win=WINDOW):
    import concourse.mybir as mybir
    import concourse.tile as tile
    from concourse import bacc

    dt = mybir.dt
    f32 = dt.float32
    bf16 = dt.bfloat16
    AF = mybir.ActivationFunctionType
    ALU = mybir.AluOpType

    NT = T_ // P          # token tiles
    KT = C_ // P          # contraction tiles
    WT = win // P         # window tiles
    ISQ = 1.0 / float(np.sqrt(HD))

    nc = bacc.Bacc()

    xT = nc.declare_dram_parameter("xT", [C_, T_], bf16, isOutput=False)
    wq = nc.declare_dram_parameter("wq", [C_, GH * HD], bf16, isOutput=False)
    wk = nc.declare_dram_parameter("wk", [C_, HD], bf16, isOutput=False)
    wv = nc.declare_dram_parameter("wv", [C_, HD], bf16, isOutput=False)
    wg = nc.declare_dram_parameter("wg", [GATE_CH, 1], bf16, isOutput=False)
    ve2 = nc.declare_dram_parameter("ve2", [T_, HD], bf16, isOutput=False)
    wo = nc.declare_dram_parameter("wo", [GH * HD, C_], bf16, isOutput=False)
    ccd = nc.declare_dram_parameter("cc", [P, T_], bf16, isOutput=False)
    ssd = nc.declare_dram_parameter("ss", [P, T_], bf16, isOutput=False)
    tlo = nc.declare_dram_parameter("tlo", [P, P], bf16, isOutput=False)
    thi = nc.declare_dram_parameter("thi", [P, P], bf16, isOutput=False)
    idr = nc.declare_dram_parameter("identr", [P, GH * P], bf16, isOutput=False)
    idf = nc.declare_dram_parameter("identf", [P, P], f32, isOutput=False)
    out_d = nc.declare_dram_parameter("out", [T_, C_], f32, isOutput=True)
    f32r = dt.float32r

    with tile.TileContext(nc) as tc:
        with (
            tc.tile_pool(name="singles", bufs=1) as sg,
            tc.tile_pool(name="work", bufs=2) as wk_pool,
            tc.tile_pool(name="work3", bufs=4) as w3_pool,
            tc.tile_pool(name="attw", bufs=4) as aw,
            tc.tile_pool(name="yup", bufs=3) as yu_pool,
            tc.tile_pool(name="outp", bufs=3) as op_pool,
            tc.tile_pool(name="psum", bufs=8, space="PSUM") as pp,
        ):
            # ---- persistent inputs -------------------------------------
            # weight/x DMAs are split per k-tile and interleaved so the
            # first projection matmuls (kt=0) can start almost immediately
            # small constants FIRST (the rope tails read cc/ss early — they
            # must not queue behind the 13MB of x/weight traffic)
            wg_sb = sg.tile([GATE_CH, 1], bf16, tag="wg")
            nc.sync.dma_start(out=wg_sb[:], in_=wg[:])
            cc_sb = sg.tile([P, T_], bf16, tag="cc")
            nc.sync.dma_start(out=cc_sb[:], in_=ccd[:])
            ss_sb = sg.tile([P, T_], bf16, tag="ss")
            nc.sync.dma_start(out=ss_sb[:], in_=ssd[:])
            ve2_sb = sg.tile([P, NT, HD], bf16, tag="ve2")
            nc.sync.dma_start(out=ve2_sb[:], in_=ve2.rearrange("(o p) d -> p o d", p=P))
            tlo_sb = sg.tile([P, P], bf16, tag="tlo")
            nc.sync.dma_start(out=tlo_sb[:], in_=tlo[:])
            thi_sb = sg.tile([P, P], bf16, tag="thi")
            nc.sync.dma_start(out=thi_sb[:], in_=thi[:])
            idr_sb = sg.tile([P, GH * P], bf16, tag="idr")
            nc.sync.dma_start(out=idr_sb[:], in_=idr[:])
            idf_sb = sg.tile([P, P], f32, tag="idf")
            nc.sync.dma_start(out=idf_sb[:], in_=idf[:])
            xt = []
            wq_sb = sg.tile([P, KT, GH * HD], bf16, tag="wq")
            wk_sb = sg.tile([P, KT, HD], bf16, tag="wk")
            wv_sb = sg.tile([P, KT, HD], bf16, tag="wv")
            wqr = wq.rearrange("(o p) n -> p o n", p=P)
            wkr = wk.rearrange("(o p) n -> p o n", p=P)
            wvr = wv.rearrange("(o p) n -> p o n", p=P)
            for kt in range(KT):
                t_ = sg.tile([P, T_], bf16, tag=f"xt{kt}")
                nc.sync.dma_start(out=wk_sb[:, kt, :], in_=wkr[:, kt, :])
                nc.sync.dma_start(out=t_[:], in_=xT[kt * P:(kt + 1) * P, :])
                xt.append(t_)
                nc.sync.dma_start(out=wq_sb[:, kt, :], in_=wqr[:, kt, :])
                nc.sync.dma_start(out=wv_sb[:, kt, :], in_=wvr[:, kt, :])
            wo_sb = sg.tile([P, GH, C_], bf16, tag="wo")
            nc.sync.dma_start(out=wo_sb[:], in_=wo.rearrange("(o p) n -> p o n", p=P))
            ones_sb = sg.tile([P, 1], bf16, tag="onesb")
            nc.vector.memset(ones_sb[:], 1.0)
            ones1f = sg.tile([1, P], f32, tag="ones1f")
            nc.vector.memset(ones1f[:], 1.0)
            eps_sb = sg.tile([P, 1], f32, tag="epsb")
            nc.vector.memset(eps_sb[:], 1e-6)

            # persistent intermediates
            qhat = sg.tile([P, GH, T_], bf16, tag="qhat")   # normalized roped q, [d, h, t]
            khat = sg.tile([P, T_], bf16, tag="khat")       # normalized roped k * isq
            vsb = sg.tile([P, NT, HD], bf16, tag="vsb")     # gated v, [tok, tt, d]

            TS = T_ // 512  # 512-wide token slices

            # ---- projections + rope + rmsnorm for k/q heads and vT -----
            # Emitted as kt-major WAVES of 3 output groups: the PE chases the
            # xT DMAs tile-by-tile during the ramp, and each wave's dependent
            # tail work (rope/rms/broadcast) is batched behind the next
            # wave's matmuls so the PE stream never waits on DVE/ACT chains.
            def wave_mms(wave):
                items = []
                for (head, ts_) in wave:
                    sl = slice(ts_ * 512, ts_ * 512 + 512)
                    ps = pp.tile([P, 512], f32, tag="pb",
                                 name=f"ps{head}_{ts_}")
                    items.append((head, sl, ps))
                for kt in range(KT):
                    for gi, (head, ts_) in enumerate(wave):
                        if head == 0:
                            w_ap = wk_sb[:, kt, :]
                        elif head == GH + 1:
                            w_ap = wv_sb[:, kt, :]
                        else:
                            w_ap = wq_sb[:, kt, (head - 1) * HD:head * HD]
                        nc.tensor.matmul(
                            items[gi][2][:], lhsT=w_ap,
                            rhs=xt[kt][:, items[gi][1]],
                            start=(kt == 0), stop=(kt == KT - 1),
                        )
                return items

            def v_tail(head, sl, ps):
                # vT psum [d, tok] -> sbuf f32, then PE-transpose each 128-tok
                # block to natural [tok, d] and add the sigmoid-gated ve.
                vt = wk_pool.tile([P, 512], f32, tag="vt")
                nc.vector.tensor_copy(vt[:], ps[:])
                for i in range(4):
                    tt = sl.start // P + i
                    tsl = slice(tt * P, (tt + 1) * P)
                    tp = pp.tile([P, P], f32, tag="pb")
                    nc.tensor.transpose(tp[:], vt[:, i * P:(i + 1) * P], idf_sb[:])
                    gps = pp.tile([P, 1], f32, tag="pb")
                    nc.tensor.matmul(gps[:], lhsT=xt[0][0:GATE_CH, tsl],
                                     rhs=wg_sb[:], start=True, stop=True)
                    gcol = wk_pool.tile([P, 1], f32, tag="gcol")
                    nc.scalar.activation(gcol[:], gps[:], AF.Sigmoid)
                    # v = ve2 * sigmoid(g) + v_proj (ve2 pre-scaled by 2)
                    nc.vector.scalar_tensor_tensor(
                        out=vsb[:, tt, :], in0=ve2_sb[:, tt, :], scalar=gcol[:],
                        in1=tp[:], op0=ALU.mult, op1=ALU.add,
                    )

            def wave_tails(items):
                t1 = []
                for (head, sl, ps) in items:
                    if head == GH + 1:
                        v_tail(head, sl, ps)
                        continue
                    # rope: qr = ps*cc + swap(ps)*ss  (ss carries the sign)
                    qr = w3_pool.tile([P, 512], f32, tag="qr")
                    nc.vector.tensor_mul(qr[:], ps[:], cc_sb[:, sl])
                    qs = wk_pool.tile([P, 512], f32, tag="qs")
                    nc.vector.tensor_mul(qs[0:64, :], ps[64:128, :],
                                         ss_sb[0:64, sl])
                    nc.vector.tensor_mul(qs[64:128, :], ps[0:64, :],
                                         ss_sb[64:128, sl])
                    nc.vector.tensor_add(qr[:], qr[:], qs[:])
                    q2 = wk_pool.tile([P, 512], bf16, tag="q2")
                    nc.gpsimd.tensor_mul(q2[:], qr[:], qr[:])
                    t1.append((head, sl, qr, q2))
                ssqs = []
                for (head, sl, qr, q2) in t1:
                    ssq = pp.tile([1, 512], f32, tag="pb")
                    nc.tensor.matmul(ssq[:], lhsT=ones_sb[:], rhs=q2[:],
                                     start=True, stop=True)
                    ssqs.append(ssq)
                rows = []
                for (head, sl, qr, q2), ssq in zip(t1, ssqs):
                    srow = w3_pool.tile([1, 512], f32, tag="srow")
                    nc.scalar.activation(srow[:], ssq[:], AF.Sqrt,
                                         bias=eps_sb[0:1, :], scale=1.0 / HD)
                    rows.append(srow)
                rrs = []
                for (head, sl, qr, q2), srow in zip(t1, rows):
                    rr = w3_pool.tile([1, 512], f32, tag="rr")
                    nc.vector.reciprocal_approx_fast(rr[:], srow[:])
                    if head == 0:
                        # fold the 1/sqrt(d) score scale into k-hat
                        nc.vector.tensor_scalar_mul(rr[:], rr[:], ISQ)
                    rrs.append(rr)
                rrbs = []
                for (head, sl, qr, q2), rr in zip(t1, rrs):
                    rrb = pp.tile([P, 512], f32, tag="pb")
                    nc.tensor.matmul(rrb[:], lhsT=ones1f[:], rhs=rr[:],
                                     start=True, stop=True)
                    rrbs.append(rrb)
                for (head, sl, qr, q2), rrb in zip(t1, rrbs):
                    dest = khat[:, sl] if head == 0 else qhat[:, head - 1, sl]
                    nc.vector.tensor_mul(dest, qr[:], rrb[:])

            groups = [(head, ts_) for head in range(GH + 2)
                      for ts_ in range(TS)]
            prev_items = None
            for w0 in range(0, len(groups), 3):
                items = wave_mms(groups[w0:w0 + 3])
                if prev_items:
                    wave_tails(prev_items)
                prev_items = items
            wave_tails(prev_items)

            CO = C_ // 512  # output column chunks
            # All 4 q-heads are fused into one 512-wide moving operand:
            # scores / exp / den / PV are each ONE N=512 instruction per
            # (qi, kt), so LDWEIGHTS fully hides under the matmul stream.
            denps = {}
            yps = {}
            yus = {}
            rds = {}

            def attn_scores_k(qi, kk):
                ktc = min(WT + 1, NT - qi)
                qs4 = qhat[:, :, qi * P:(qi + 1) * P]   # [d, (h, q)] = 512 wide
                kt = qi + kk
                sp = pp.tile([P, GH * P], f32, tag="pb")
                masked = (kk == 0) or (kk == WT and ktc == WT + 1)
                nc.tensor.matmul(
                    sp[:], lhsT=khat[:, kt * P:(kt + 1) * P], rhs=qs4,
                    start=True, stop=not masked,
                )
                if masked:
                    # band-mask bias (-3e4 outside band): psum += bias.T @ I_rep
                    nc.tensor.matmul(
                        sp[:], lhsT=tlo_sb[:] if kk == 0 else thi_sb[:],
                        rhs=idr_sb[:], start=False, stop=True,
                    )
                pt = aw.tile([P, GH * P], bf16, tag="pT")
                nc.scalar.activation(pt[:], sp[:], AF.Exp)
                return pt

            def attn_pv_k(qi, kk, pt):
                ktc = min(WT + 1, NT - qi)
                if kk == 0:
                    denps[qi] = pp.tile([1, GH * P], f32, tag="pb",
                                        name=f"denp{qi}")
                    yps[qi] = pp.tile([P, GH * P], f32, tag="pb",
                                      name=f"yp{qi}")
                kt = qi + kk
                nc.tensor.matmul(
                    denps[qi][:], lhsT=ones_sb[:], rhs=pt[:],
                    start=(kk == 0), stop=(kk == ktc - 1),
                )
                nc.tensor.matmul(
                    yps[qi][:], lhsT=vsb[:, kt, :], rhs=pt[:],
                    start=(kk == 0), stop=(kk == ktc - 1),
                )
                if kk == ktc - 1:
                    yut = yu_pool.tile([P, GH * P], f32, tag="yu")
                    nc.vector.tensor_copy(yut[:], yps[qi][:])
                    yus[qi] = yut
                    rd = wk_pool.tile([1, GH * P], f32, tag="rd")
                    nc.vector.reciprocal_approx_fast(rd[:], denps[qi][:])
                    rds[qi] = rd

            def attn_out(qi):
                qsl = slice(qi * P, (qi + 1) * P)
                rdb = pp.tile([P, GH * P], f32, tag="pb")
                nc.tensor.matmul(rdb[:], lhsT=ones1f[:], rhs=rds[qi][:],
                                 start=True, stop=True)
                yq = op_pool.tile([P, GH * P], bf16, tag="yq")
                nc.vector.tensor_mul(yq[:], yus[qi][:], rdb[:])
                for co in range(CO):
                    osl = slice(co * 512, co * 512 + 512)
                    ops = pp.tile([P, 512], f32, tag="pb")
                    for h in range(GH):
                        nc.tensor.matmul(
                            ops[:], lhsT=yq[:, h * P:(h + 1) * P],
                            rhs=wo_sb[:, h, osl],
                            start=(h == 0), stop=(h == GH - 1),
                        )
                    ob = op_pool.tile([P, 512], f32, tag="ob")
                    nc.vector.tensor_copy(out=ob[:], in_=ops[:])
                    nc.sync.dma_start(out=out_d[qsl, osl], in_=ob[:])

            pv_queue = deque()
            done_out = set()
            out_ready = deque()
            for qi in range(NT):
                ktc = min(WT + 1, NT - qi)
                for kk in range(ktc):
                    pt = attn_scores_k(qi, kk)
                    if len(pv_queue) >= 2:
                        attn_pv_k(*pv_queue.popleft())
                    pv_queue.append((qi, kk, pt))
                    # emit out-proj one iteration after its recip is queued,
                    # so the PE never waits on the denominator chain
                    if out_ready and out_ready[0][1] <= 0:
                        done_out.add(out_ready[0][0])
                        attn_out(out_ready.popleft()[0])
                    out_ready = deque([(q, age - 1) for q, age in out_ready])
                    if qi > 0 and (qi - 1) in rds and (qi - 1) not in done_out \
                            and all(q != qi - 1 for q, _ in out_ready):
                        out_ready.append((qi - 1, 1))
            while pv_queue:
                attn_pv_k(*pv_queue.popleft())
            for qi in range(NT):
                if qi not in done_out:
                    attn_out(qi)

    return nc


def _get_program(T_=T, C_=C, win=WINDOW):
    key = (T_, C_, win)
    if key not in _PROGRAM_CACHE:
        nc = build_program(T_, C_, win)
        nc.finalize()
        _PROGRAM_CACHE[key] = nc
    return _PROGRAM_CACHE[key]


def make_in_maps(x, ve, cos, sin, Wq, Wk, Wv, Wg, Wo):
    """Build the 8 per-core input dicts (host-side sharding/layout prep)."""
    cosT = np.ascontiguousarray(cos[:, 0, :].T).astype(np.float32)  # [64, T]
    sinT = np.ascontiguousarray(sin[:, 0, :].T).astype(np.float32)
    cc = np.concatenate([cosT, cosT], axis=0)            # [128, T]
    ss = np.concatenate([sinT, -sinT], axis=0)           # [128, T]
    # additive mask biases for the S^T diagonal/far tiles, pre-transposed
    # (they enter the scores as lhsT with an identity rhs: psum += bias.T)
    neg = np.float32(-30000.0)
    bias_lo = np.where(np.arange(P)[:, None] >= np.arange(P)[None, :], 0.0, neg)
    bias_hi = np.where(np.arange(P)[:, None] < np.arange(P)[None, :], 0.0, neg)
    tlo = np.ascontiguousarray(bias_lo.T).astype(BF16)
    thi = np.ascontiguousarray(bias_hi.T).astype(BF16)
    identr = np.tile(np.eye(P, dtype=np.float32), (1, GH)).astype(BF16)
    identf = np.eye(P, dtype=np.float32)

    in_maps = []
    for core in range(N_CORES):
        b, g = divmod(core, N_KV)
        in_maps.append({
            "xT": np.ascontiguousarray(x[b].T).astype(BF16),
            "wq": Wq[:, g * GH * HD:(g + 1) * GH * HD].astype(BF16),
            "wk": Wk[:, g * HD:(g + 1) * HD].astype(BF16),
            "wv": Wv[:, g * HD:(g + 1) * HD].astype(BF16),
            "wg": np.ascontiguousarray(Wg[:, g:g + 1]).astype(BF16),
            "ve2": (2.0 * ve[b][:, g * HD:(g + 1) * HD]).astype(BF16),
            "wo": Wo[g * GH * HD:(g + 1) * GH * HD, :].astype(BF16),
            "cc": cc.astype(BF16), "ss": ss.astype(BF16),
            "tlo": tlo, "thi": thi, "identr": identr, "identf": identf,
        })
    return in_maps


def kernel(x, ve, cos, sin, Wq, Wk, Wv, Wg, Wo, window):
    assert int(window) == WINDOW and x.shape == (B, T, C)
    from concourse.bass_utils import run_bass_kernel_spmd

    nc = _get_program()
    in_maps = make_in_maps(x, ve, cos, sin, Wq, Wk, Wv, Wg, Wo)
    res = run_bass_kernel_spmd(nc, in_maps, core_ids=list(range(N_CORES)))
    out = np.zeros((B, T, C), dtype=np.float32)
    for core in range(N_CORES):
        b = core // N_KV
        out[b] += res.results[core]["out"]
    return out
